# revision 8
# baseline (speedup 1.0000x reference)
"""Bass/Tile TRN2 kernel for nn_Actor_DeepSet (8-core data parallel).

Reference computation (per row r = b*8 + i, obs=64, hidden=128):
  h1   = relu(x_r @ w1.T + b1)
  hsum = (1/8) * sum_{k=1..7} relu(rot_{i+1}(x_{b,k}) @ w1o.T + b1o)
  h2   = relu([h1, hsum] @ w2.T + b2)
  out  = h2 @ wv.T + bv
rot_s rotates the 64 features; equivalently a column rotation of w1o.  The
1/8 folds into w1o/b1o (relu positively homogeneous).

Device layout: transposed (channels on partitions, rows on free axis), bf16
in / f32 PSUM.  Each 512-row tile is reordered agent-major on the host
(tile column j = a*64 + b) so every matmul moving operand is contiguous.
Tiles are processed in PAIRS sharing 2-bank PSUM tiles, so each drain /
TT-add instruction covers both tiles (halves per-op overhead), and matmuls
are grouped by stationary operand to share LDWEIGHTS.  Layer-1 "other"
products are stored relu'd in SBUF as r[128, k=7, t=2, s=8, b=64]; the k-sum
runs partly as DVE adds over [128,1024] slabs and partly folded into the
layer-2 PSUM accumulation.  Biases are applied by the drain ops.
Output y.T [16, 16384] in tile-(a,b) order; host unscrambles.
"""

import os
import numpy as np

import concourse.bacc as bacc
import concourse.mybir as mybir
import concourse.tile as tile
from concourse.bass_utils import run_bass_kernel_spmd

N_CORES = 8
N_AGENTS = 8
OBS = 64
HIDDEN = 128
NUM_OUT = 16
ROWS_PC = 16384
TILE_N = 512
N_TILES = ROWS_PC // TILE_N
NB = TILE_N // N_AGENTS
PAIR_N = 2 * TILE_N  # 1024

# tuning knobs
N_FOLD = int(os.environ.get("KN_FOLD", "2"))
DVE_SHIFT_SET = set(int(c) for c in os.environ.get("KN_DVE_SET", "146"))
HTOP_ON_ACT = bool(int(os.environ.get("KN_HTOP_ACT", "0")))
H2_ON_ACT = bool(int(os.environ.get("KN_H2_ACT", "1")))
O_ON_ACT = bool(int(os.environ.get("KN_O_ACT", "1")))

BF16 = mybir.dt.bfloat16
F32 = mybir.dt.float32
NP_BF16 = mybir.dt.np(BF16)
ALU = mybir.AluOpType
AF = mybir.ActivationFunctionType

_compiled_nc = None
last_exec_time_ns = None


def _build_nc():
    nc = bacc.Bacc("TRN2", target_bir_lowering=False, debug=False,
                   num_devices=N_CORES)

    x_ext = nc.dram_tensor("x", [OBS, ROWS_PC], BF16, kind="ExternalInput")
    wl1_ext = nc.dram_tensor("wl1", [OBS, HIDDEN], BF16, kind="ExternalInput")
    wcat_ext = nc.dram_tensor("wcat", [N_AGENTS, OBS, HIDDEN], BF16,
                              kind="ExternalInput")
    w2a_ext = nc.dram_tensor("w2a", [HIDDEN, HIDDEN], BF16, kind="ExternalInput")
    w2b_ext = nc.dram_tensor("w2b", [HIDDEN, HIDDEN], BF16, kind="ExternalInput")
    wv_ext = nc.dram_tensor("wv", [HIDDEN, NUM_OUT], BF16, kind="ExternalInput")
    b1_ext = nc.dram_tensor("b1", [HIDDEN, 1], F32, kind="ExternalInput")
    b1o_ext = nc.dram_tensor("b1o", [HIDDEN, 1], F32, kind="ExternalInput")
    b2_ext = nc.dram_tensor("b2", [HIDDEN, 1], F32, kind="ExternalInput")
    bv_ext = nc.dram_tensor("bv", [NUM_OUT, 1], F32, kind="ExternalInput")
    y_ext = nc.dram_tensor("y", [NUM_OUT, ROWS_PC], F32, kind="ExternalOutput")

    with tile.TileContext(nc) as tc:
        with (
            tc.tile_pool(name="const", bufs=1) as cpool,
            tc.tile_pool(name="xin", bufs=4) as xpool,
            tc.tile_pool(name="act", bufs=3) as apool,
            tc.tile_pool(name="rbuf", bufs=3) as rpool,
            tc.tile_pool(name="outb", bufs=3) as opool,
            tc.tile_pool(name="ps", bufs=4, space="PSUM") as pps,
        ):
            # --- persistent weights / biases ---
            wl1 = cpool.tile([OBS, HIDDEN], BF16)
            nc.sync.dma_start(wl1[:], wl1_ext[:])
            wcat = cpool.tile([OBS, N_AGENTS * HIDDEN], BF16)
            for s in range(N_AGENTS):
                nc.sync.dma_start(wcat[:, s * HIDDEN:(s + 1) * HIDDEN],
                                  wcat_ext[s])
            w2a = cpool.tile([HIDDEN, HIDDEN], BF16)
            nc.sync.dma_start(w2a[:], w2a_ext[:])
            w2b = cpool.tile([HIDDEN, HIDDEN], BF16)
            nc.sync.dma_start(w2b[:], w2b_ext[:])
            wv = cpool.tile([HIDDEN, NUM_OUT], BF16)
            nc.sync.dma_start(wv[:], wv_ext[:])
            b1t = cpool.tile([HIDDEN, 1], F32)
            nc.sync.dma_start(b1t[:], b1_ext[:])
            b1ot = cpool.tile([HIDDEN, 1], F32)
            nc.sync.dma_start(b1ot[:], b1o_ext[:])
            b2t = cpool.tile([HIDDEN, 1], F32)
            nc.sync.dma_start(b2t[:], b2_ext[:])
            bvt = cpool.tile([NUM_OUT, 1], F32)
            nc.sync.dma_start(bvt[:], bv_ext[:])

            n_tt = 6 - N_FOLD

            def drain(dst, src, bias, on_act):
                """relu(src + bias) -> dst (PSUM -> SBUF)."""
                if on_act:
                    nc.scalar.activation(dst, src, AF.Relu, bias=bias)
                else:
                    nc.vector.tensor_scalar(dst, src, bias, 0.0,
                                            ALU.add, ALU.max)

            for pair in range(N_TILES // 2):
                t0 = 2 * pair
                col0 = t0 * TILE_N

                xt = xpool.tile([OBS, PAIR_N], BF16)
                nc.sync.dma_start(xt[:], x_ext[:, col0:col0 + PAIR_N])

                # --- layer-1 self (shared wl1 stationary; 2-bank psum) ---
                ps1 = pps.tile([HIDDEN, PAIR_N], F32, tag="ps")
                nc.tensor.matmul(ps1[:, :TILE_N], wl1[:], xt[:, :TILE_N])
                nc.tensor.matmul(ps1[:, TILE_N:], wl1[:], xt[:, TILE_N:])
                htop = apool.tile([HIDDEN, PAIR_N], BF16, tag="htop")
                drain(htop[:], ps1[:], b1t[:], HTOP_ON_ACT)

                # --- layer-1 others ---
                # r layout [128, k=7, t=2, s=8, b=64]; shift-s psum pair holds
                # (t0, t1) k-major blocks; drain src iterates (k, t, b).
                r = rpool.tile([HIDDEN, 7 * PAIR_N], BF16)
                r_v = r[:].rearrange("p (k t s b) -> p k t s b",
                                     k=7, t=2, s=N_AGENTS)
                for s in range(N_AGENTS):
                    w_s = wcat[:, s * HIDDEN:(s + 1) * HIDDEN]
                    ps = pps.tile([HIDDEN, PAIR_N], F32, tag="ps")
                    for ti in range(2):
                        nc.tensor.matmul(
                            ps[:, ti * TILE_N:ti * TILE_N + 7 * NB], w_s,
                            xt[:, ti * TILE_N + NB:(ti + 1) * TILE_N])
                    src = ps[:].rearrange("p (t c) -> p t c", t=2)
                    src = src[:, :, :7 * NB].rearrange("p t (k b) -> p k t b",
                                                       k=7)
                    drain(r_v[:, :, :, s, :], src, b1ot[:], s not in DVE_SHIFT_SET)

                # --- partial k-sums on DVE ([128, 1024] slabs) ---
                r_k = r[:].rearrange("p (k c) -> p k c", k=7)
                hbot = apool.tile([HIDDEN, PAIR_N], BF16, tag="hbot")
                with nc.allow_low_precision("bf16 partial sums"):
                    if n_tt == 0:
                        hbot = None
                    elif n_tt >= 3:
                        tmp = apool.tile([HIDDEN, PAIR_N], BF16, tag="tmp")
                        nc.vector.tensor_add(hbot[:], r_k[:, 0, :], r_k[:, 1, :])
                        nc.vector.tensor_add(tmp[:], r_k[:, 2, :], r_k[:, 3, :])
                        for k in range(4, n_tt + 1):
                            nc.vector.tensor_add(tmp[:], tmp[:], r_k[:, k, :])
                        nc.vector.tensor_add(hbot[:], hbot[:], tmp[:])
                    else:
                        nc.vector.tensor_add(hbot[:], r_k[:, 0, :], r_k[:, 1, :])
                        for k in range(2, n_tt + 1):
                            nc.vector.tensor_add(hbot[:], hbot[:], r_k[:, k, :])

                # --- layer 2 (grouped by stationary; folds ride the psum) ---
                ps2 = pps.tile([HIDDEN, PAIR_N], F32, tag="ps")
                for ti in range(2):
                    sl = slice(ti * TILE_N, (ti + 1) * TILE_N)
                    nc.tensor.matmul(ps2[:, sl], w2a[:], htop[:, sl],
                                     start=True, stop=False)
                first_fold = 7 - N_FOLD if n_tt > 0 else 0
                r_kt = r[:].rearrange("p (k t c) -> p k t c", k=7, t=2)
                for ti in range(2):
                    sl = slice(ti * TILE_N, (ti + 1) * TILE_N)
                    if hbot is not None:
                        nc.tensor.matmul(ps2[:, sl], w2b[:],
                                         hbot[:, sl],
                                         start=False, stop=(N_FOLD == 0))
                    for k in range(first_fold, 7):
                        nc.tensor.matmul(ps2[:, sl], w2b[:], r_kt[:, k, ti, :],
                                         start=False, stop=(k == 6))
                h2 = apool.tile([HIDDEN, PAIR_N], BF16, tag="h2")
                drain(h2[:], ps2[:], b2t[:], H2_ON_ACT)

                # --- layer 3 (2-bank [16, 1024] psum from the shift pool) ---
                ps3 = pps.tile([NUM_OUT, PAIR_N], F32, tag="ps")
                for ti in range(2):
                    sl = slice(ti * TILE_N, (ti + 1) * TILE_N)
                    nc.tensor.matmul(ps3[:, sl], wv[:], h2[:, sl])
                o = opool.tile([NUM_OUT, PAIR_N], F32)
                if O_ON_ACT:
                    nc.scalar.activation(o[:], ps3[:], AF.Identity, bias=bvt[:])
                else:
                    nc.vector.tensor_scalar_add(o[:], ps3[:], bvt[:])
                nc.sync.dma_start(y_ext[:, col0:col0 + PAIR_N], o[:])

    nc.compile()
    return nc


def kernel(inputs, w1, b1, w1o, b1o, w2, b2, wv, bv):
    global _compiled_nc, last_exec_time_ns
    if _compiled_nc is None:
        _compiled_nc = _build_nc()
    nc = _compiled_nc

    inputs = np.asarray(inputs, dtype=np.float32)
    w1 = np.asarray(w1, dtype=np.float32)
    b1 = np.asarray(b1, dtype=np.float32)
    w1o = np.asarray(w1o, dtype=np.float32)
    b1o = np.asarray(b1o, dtype=np.float32)
    w2 = np.asarray(w2, dtype=np.float32)
    b2 = np.asarray(b2, dtype=np.float32)
    wv = np.asarray(wv, dtype=np.float32)
    bv = np.asarray(bv, dtype=np.float32)

    wl1 = np.ascontiguousarray(w1.T).astype(NP_BF16)
    wcat = np.empty((N_AGENTS, OBS, HIDDEN), dtype=NP_BF16)
    for si in range(N_AGENTS):
        wcat[si] = (np.roll(w1o, si + 1, axis=1).T / N_AGENTS).astype(NP_BF16)
    w2a = np.ascontiguousarray(w2[:, :HIDDEN].T).astype(NP_BF16)
    w2b = np.ascontiguousarray(w2[:, HIDDEN:].T).astype(NP_BF16)
    wvt = np.ascontiguousarray(wv.T).astype(NP_BF16)
    b1c = np.ascontiguousarray(b1[:, None]).astype(np.float32)
    b1oc = np.ascontiguousarray((b1o / N_AGENTS)[:, None]).astype(np.float32)
    b2c = np.ascontiguousarray(b2[:, None]).astype(np.float32)
    bvc = np.ascontiguousarray(bv[:, None]).astype(np.float32)

    xs = inputs.reshape(N_CORES, N_TILES, NB, N_AGENTS, OBS)
    xs_t = xs.transpose(0, 4, 1, 3, 2).reshape(N_CORES, OBS, ROWS_PC)
    in_maps = []
    for c in range(N_CORES):
        in_maps.append({
            "x": np.ascontiguousarray(xs_t[c]).astype(NP_BF16),
            "wl1": wl1, "wcat": wcat, "w2a": w2a, "w2b": w2b, "wv": wvt,
            "b1": b1c, "b1o": b1oc, "b2": b2c, "bv": bvc,
        })

    trace = bool(int(os.environ.get("BASS_KERNEL_TRACE", "0")))
    res = run_bass_kernel_spmd(nc, in_maps, list(range(N_CORES)), trace=trace)
    last_exec_time_ns = res.exec_time_ns

    y = np.stack([res.results[c]["y"] for c in range(N_CORES)])
    y = y.reshape(N_CORES, NUM_OUT, N_TILES, N_AGENTS, NB)
    out = y.transpose(0, 2, 4, 3, 1).reshape(N_CORES * ROWS_PC, NUM_OUT)
    return np.ascontiguousarray(out, dtype=np.float32)


# revision 9
# speedup vs baseline: 1.4270x; 1.4270x over previous
"""Bass/Tile TRN2 kernel for nn_Actor_DeepSet (8-core data parallel).

Reference computation (per row r = b*8 + i, obs=64, hidden=128):
  h1   = relu(x_r @ w1.T + b1)
  hsum = (1/8) * sum_{k=1..7} relu(rot_{i+1}(x_{b,k}) @ w1o.T + b1o)
  h2   = relu([h1, hsum] @ w2.T + b2)
  out  = h2 @ wv.T + bv
rot_s rotates the 64 features; equivalently a column rotation of w1o.  The
1/8 folds into w1o/b1o (relu positively homogeneous).

Device layout: transposed (channels on partitions, rows on free axis), bf16
in / f32 PSUM.  Each 512-row tile is reordered agent-major on the host
(tile column j = a*64 + b) so every matmul moving operand is contiguous.
Tiles are processed in PAIRS sharing 2-bank PSUM tiles, so each drain /
TT-add instruction covers both tiles (halves per-op overhead), and matmuls
are grouped by stationary operand to share LDWEIGHTS.  Layer-1 "other"
products are stored relu'd in SBUF as r[128, k=7, t=2, s=8, b=64]; the k-sum
runs partly as DVE adds over [128,1024] slabs and partly folded into the
layer-2 PSUM accumulation.  Biases are applied by the drain ops.
Output y.T [16, 16384] in tile-(a,b) order; host unscrambles.
"""

import os
import numpy as np

import concourse.bacc as bacc
import concourse.mybir as mybir
import concourse.tile as tile
from concourse.bass_utils import run_bass_kernel_spmd

N_CORES = 8
N_AGENTS = 8
OBS = 64
HIDDEN = 128
NUM_OUT = 16
ROWS_PC = 16384
TILE_N = 512
N_TILES = ROWS_PC // TILE_N
NB = TILE_N // N_AGENTS
PAIR_N = 2 * TILE_N  # 1024

# tuning knobs
N_FOLD = int(os.environ.get("KN_FOLD", "2"))
DVE_SHIFT_SET = set(int(c) for c in os.environ.get("KN_DVE_SET", "146"))
HTOP_ON_ACT = bool(int(os.environ.get("KN_HTOP_ACT", "0")))
H2_ON_ACT = bool(int(os.environ.get("KN_H2_ACT", "1")))
O_ON_ACT = bool(int(os.environ.get("KN_O_ACT", "1")))

BF16 = mybir.dt.bfloat16
F32 = mybir.dt.float32
NP_BF16 = mybir.dt.np(BF16)
ALU = mybir.AluOpType
AF = mybir.ActivationFunctionType

_compiled_nc = None
last_exec_time_ns = None


def _build_nc():
    nc = bacc.Bacc("TRN2", target_bir_lowering=False, debug=False,
                   num_devices=N_CORES)

    x_ext = nc.dram_tensor("x", [OBS, ROWS_PC], BF16, kind="ExternalInput")
    wl1_ext = nc.dram_tensor("wl1", [OBS, HIDDEN], BF16, kind="ExternalInput")
    wcat_ext = nc.dram_tensor("wcat", [N_AGENTS, OBS, HIDDEN], BF16,
                              kind="ExternalInput")
    w2a_ext = nc.dram_tensor("w2a", [HIDDEN, HIDDEN], BF16, kind="ExternalInput")
    w2b_ext = nc.dram_tensor("w2b", [HIDDEN, HIDDEN], BF16, kind="ExternalInput")
    wv_ext = nc.dram_tensor("wv", [HIDDEN, NUM_OUT], BF16, kind="ExternalInput")
    b1_ext = nc.dram_tensor("b1", [HIDDEN, 1], F32, kind="ExternalInput")
    b1o_ext = nc.dram_tensor("b1o", [HIDDEN, 1], F32, kind="ExternalInput")
    b2_ext = nc.dram_tensor("b2", [HIDDEN, 1], F32, kind="ExternalInput")
    bv_ext = nc.dram_tensor("bv", [NUM_OUT, 1], F32, kind="ExternalInput")
    y_ext = nc.dram_tensor("y", [NUM_OUT, ROWS_PC], F32, kind="ExternalOutput")

    with tile.TileContext(nc) as tc:
        with (
            tc.tile_pool(name="const", bufs=1) as cpool,
            tc.tile_pool(name="xin", bufs=4) as xpool,
            tc.tile_pool(name="act", bufs=3) as apool,
            tc.tile_pool(name="rbuf", bufs=3) as rpool,
            tc.tile_pool(name="outb", bufs=3) as opool,
            tc.tile_pool(name="ps", bufs=4, space="PSUM") as pps,
        ):
            # --- persistent weights / biases ---
            wl1 = cpool.tile([OBS, HIDDEN], BF16)
            nc.sync.dma_start(wl1[:], wl1_ext[:])
            wcat = cpool.tile([OBS, N_AGENTS * HIDDEN], BF16)
            for s in range(N_AGENTS):
                nc.sync.dma_start(wcat[:, s * HIDDEN:(s + 1) * HIDDEN],
                                  wcat_ext[s])
            w2a = cpool.tile([HIDDEN, HIDDEN], BF16)
            nc.sync.dma_start(w2a[:], w2a_ext[:])
            w2b = cpool.tile([HIDDEN, HIDDEN], BF16)
            nc.sync.dma_start(w2b[:], w2b_ext[:])
            wv = cpool.tile([HIDDEN, NUM_OUT], BF16)
            nc.sync.dma_start(wv[:], wv_ext[:])
            b1t = cpool.tile([HIDDEN, 1], F32)
            nc.sync.dma_start(b1t[:], b1_ext[:])
            b1ot = cpool.tile([HIDDEN, 1], F32)
            nc.sync.dma_start(b1ot[:], b1o_ext[:])
            b2t = cpool.tile([HIDDEN, 1], F32)
            nc.sync.dma_start(b2t[:], b2_ext[:])
            bvt = cpool.tile([NUM_OUT, 1], F32)
            nc.sync.dma_start(bvt[:], bv_ext[:])

            n_tt = 6 - N_FOLD

            def drain(dst, src, bias, on_act):
                """relu(src + bias) -> dst (PSUM -> SBUF)."""
                if on_act:
                    nc.scalar.activation(dst, src, AF.Relu, bias=bias)
                else:
                    nc.vector.tensor_scalar(dst, src, bias, 0.0,
                                            ALU.add, ALU.max)

            def front(pair):
                """xt DMA, layer-1 matmuls + drains, k-sum TT tree."""
                t0 = 2 * pair
                col0 = t0 * TILE_N

                xt = xpool.tile([OBS, PAIR_N], BF16)
                nc.sync.dma_start(xt[:], x_ext[:, col0:col0 + PAIR_N])

                ps1 = pps.tile([HIDDEN, PAIR_N], F32, tag="ps")
                nc.tensor.matmul(ps1[:, :TILE_N], wl1[:], xt[:, :TILE_N])
                nc.tensor.matmul(ps1[:, TILE_N:], wl1[:], xt[:, TILE_N:])
                htop = apool.tile([HIDDEN, PAIR_N], BF16, tag="htop")
                drain(htop[:], ps1[:], b1t[:], HTOP_ON_ACT)

                # r layout [128, k=7, t=2, s=8, b=64]
                r = rpool.tile([HIDDEN, 7 * PAIR_N], BF16)
                r_v = r[:].rearrange("p (k t s b) -> p k t s b",
                                     k=7, t=2, s=N_AGENTS)
                for s in range(N_AGENTS):
                    w_s = wcat[:, s * HIDDEN:(s + 1) * HIDDEN]
                    ps = pps.tile([HIDDEN, PAIR_N], F32, tag="ps")
                    for ti in range(2):
                        nc.tensor.matmul(
                            ps[:, ti * TILE_N:ti * TILE_N + 7 * NB], w_s,
                            xt[:, ti * TILE_N + NB:(ti + 1) * TILE_N])
                    src = ps[:].rearrange("p (t c) -> p t c", t=2)
                    src = src[:, :, :7 * NB].rearrange("p t (k b) -> p k t b",
                                                       k=7)
                    drain(r_v[:, :, :, s, :], src, b1ot[:],
                          s not in DVE_SHIFT_SET)

                r_k = r[:].rearrange("p (k c) -> p k c", k=7)
                hbot = apool.tile([HIDDEN, PAIR_N], BF16, tag="hbot")
                with nc.allow_low_precision("bf16 partial sums"):
                    if n_tt == 0:
                        hbot = None
                    elif n_tt >= 3:
                        tmp = apool.tile([HIDDEN, PAIR_N], BF16, tag="tmp")
                        nc.vector.tensor_add(hbot[:], r_k[:, 0, :], r_k[:, 1, :])
                        nc.vector.tensor_add(tmp[:], r_k[:, 2, :], r_k[:, 3, :])
                        for k in range(4, n_tt + 1):
                            nc.vector.tensor_add(tmp[:], tmp[:], r_k[:, k, :])
                        nc.vector.tensor_add(hbot[:], hbot[:], tmp[:])
                    else:
                        nc.vector.tensor_add(hbot[:], r_k[:, 0, :], r_k[:, 1, :])
                        for k in range(2, n_tt + 1):
                            nc.vector.tensor_add(hbot[:], hbot[:], r_k[:, k, :])
                return r, hbot, htop

            def back(pair, state):
                """layer 2 + layer 3 + output DMA."""
                r, hbot, htop = state
                col0 = 2 * pair * TILE_N
                first_fold = 7 - N_FOLD if n_tt > 0 else 0
                r_kt = r[:].rearrange("p (k t c) -> p k t c", k=7, t=2)
                ps2 = pps.tile([HIDDEN, PAIR_N], F32, tag="ps")
                for ti in range(2):
                    sl = slice(ti * TILE_N, (ti + 1) * TILE_N)
                    nc.tensor.matmul(ps2[:, sl], w2a[:], htop[:, sl],
                                     start=True, stop=False)
                for ti in range(2):
                    sl = slice(ti * TILE_N, (ti + 1) * TILE_N)
                    if hbot is not None:
                        nc.tensor.matmul(ps2[:, sl], w2b[:],
                                         hbot[:, sl],
                                         start=False, stop=(N_FOLD == 0))
                    for k in range(first_fold, 7):
                        nc.tensor.matmul(ps2[:, sl], w2b[:], r_kt[:, k, ti, :],
                                         start=False, stop=(k == 6))
                h2 = apool.tile([HIDDEN, PAIR_N], BF16, tag="h2")
                drain(h2[:], ps2[:], b2t[:], H2_ON_ACT)

                ps3 = pps.tile([NUM_OUT, PAIR_N], F32, tag="ps")
                for ti in range(2):
                    sl = slice(ti * TILE_N, (ti + 1) * TILE_N)
                    nc.tensor.matmul(ps3[:, sl], wv[:], h2[:, sl])
                o = opool.tile([NUM_OUT, PAIR_N], F32)
                if O_ON_ACT:
                    nc.scalar.activation(o[:], ps3[:], AF.Identity, bias=bvt[:])
                else:
                    nc.vector.tensor_scalar_add(o[:], ps3[:], bvt[:])
                nc.sync.dma_start(y_ext[:, col0:col0 + PAIR_N], o[:])

            prev = None
            for pair in range(N_TILES // 2):
                state = front(pair)
                if prev is not None:
                    back(pair - 1, prev)
                prev = state
            back(N_TILES // 2 - 1, prev)

    nc.compile()
    return nc


def kernel(inputs, w1, b1, w1o, b1o, w2, b2, wv, bv):
    global _compiled_nc, last_exec_time_ns
    if _compiled_nc is None:
        _compiled_nc = _build_nc()
    nc = _compiled_nc

    inputs = np.asarray(inputs, dtype=np.float32)
    w1 = np.asarray(w1, dtype=np.float32)
    b1 = np.asarray(b1, dtype=np.float32)
    w1o = np.asarray(w1o, dtype=np.float32)
    b1o = np.asarray(b1o, dtype=np.float32)
    w2 = np.asarray(w2, dtype=np.float32)
    b2 = np.asarray(b2, dtype=np.float32)
    wv = np.asarray(wv, dtype=np.float32)
    bv = np.asarray(bv, dtype=np.float32)

    wl1 = np.ascontiguousarray(w1.T).astype(NP_BF16)
    wcat = np.empty((N_AGENTS, OBS, HIDDEN), dtype=NP_BF16)
    for si in range(N_AGENTS):
        wcat[si] = (np.roll(w1o, si + 1, axis=1).T / N_AGENTS).astype(NP_BF16)
    w2a = np.ascontiguousarray(w2[:, :HIDDEN].T).astype(NP_BF16)
    w2b = np.ascontiguousarray(w2[:, HIDDEN:].T).astype(NP_BF16)
    wvt = np.ascontiguousarray(wv.T).astype(NP_BF16)
    b1c = np.ascontiguousarray(b1[:, None]).astype(np.float32)
    b1oc = np.ascontiguousarray((b1o / N_AGENTS)[:, None]).astype(np.float32)
    b2c = np.ascontiguousarray(b2[:, None]).astype(np.float32)
    bvc = np.ascontiguousarray(bv[:, None]).astype(np.float32)

    xs = inputs.reshape(N_CORES, N_TILES, NB, N_AGENTS, OBS)
    xs_t = xs.transpose(0, 4, 1, 3, 2).reshape(N_CORES, OBS, ROWS_PC)
    in_maps = []
    for c in range(N_CORES):
        in_maps.append({
            "x": np.ascontiguousarray(xs_t[c]).astype(NP_BF16),
            "wl1": wl1, "wcat": wcat, "w2a": w2a, "w2b": w2b, "wv": wvt,
            "b1": b1c, "b1o": b1oc, "b2": b2c, "bv": bvc,
        })

    trace = bool(int(os.environ.get("BASS_KERNEL_TRACE", "0")))
    res = run_bass_kernel_spmd(nc, in_maps, list(range(N_CORES)), trace=trace)
    last_exec_time_ns = res.exec_time_ns

    y = np.stack([res.results[c]["y"] for c in range(N_CORES)])
    y = y.reshape(N_CORES, NUM_OUT, N_TILES, N_AGENTS, NB)
    out = y.transpose(0, 2, 4, 3, 1).reshape(N_CORES * ROWS_PC, NUM_OUT)
    return np.ascontiguousarray(out, dtype=np.float32)


# revision 10
# speedup vs baseline: 2.0176x; 1.4139x over previous
"""Bass/Tile TRN2 kernel for nn_Actor_DeepSet (8-core data parallel).

Reference computation (per row r = b*8 + i, obs=64, hidden=128):
  h1   = relu(x_r @ w1.T + b1)
  hsum = (1/8) * sum_{k=1..7} relu(rot_{i+1}(x_{b,k}) @ w1o.T + b1o)
  h2   = relu([h1, hsum] @ w2.T + b2)
  out  = h2 @ wv.T + bv
rot_s rotates the 64 features; equivalently a column rotation of w1o.  The
1/8 folds into w1o/b1o (relu positively homogeneous).

Device layout: transposed (channels on partitions, rows on free axis), bf16
in / f32 PSUM.  Each 512-row tile is reordered agent-major on the host
(tile column j = a*64 + b) so every matmul moving operand is contiguous.
Tiles are processed in PAIRS sharing 2-bank PSUM tiles, so each drain /
TT-add instruction covers both tiles (halves per-op overhead), and matmuls
are grouped by stationary operand to share LDWEIGHTS.  Layer-1 "other"
products are stored relu'd in SBUF as r[128, k=7, t=2, s=8, b=64]; the k-sum
runs partly as DVE adds over [128,1024] slabs and partly folded into the
layer-2 PSUM accumulation.  Biases are applied by the drain ops.
Output y.T [16, 16384] in tile-(a,b) order; host unscrambles.
"""

import os
import numpy as np

import concourse.bacc as bacc
import concourse.mybir as mybir
import concourse.tile as tile
from concourse.bass_utils import run_bass_kernel_spmd

N_CORES = 8
N_AGENTS = 8
OBS = 64
HIDDEN = 128
NUM_OUT = 16
ROWS_PC = 16384
TILE_N = 512
N_TILES = ROWS_PC // TILE_N
NB = TILE_N // N_AGENTS
PAIR_N = 2 * TILE_N  # 1024

# tuning knobs
N_FOLD = int(os.environ.get("KN_FOLD", "3"))
DVE_SHIFT_SET = set(int(c) for c in os.environ.get("KN_DVE_SET", "146"))
HTOP_ON_ACT = bool(int(os.environ.get("KN_HTOP_ACT", "0")))
H2_ON_ACT = bool(int(os.environ.get("KN_H2_ACT", "1")))
O_ON_ACT = bool(int(os.environ.get("KN_O_ACT", "1")))

BF16 = mybir.dt.bfloat16
F32 = mybir.dt.float32
NP_BF16 = mybir.dt.np(BF16)
ALU = mybir.AluOpType
AF = mybir.ActivationFunctionType

_compiled_nc = None
last_exec_time_ns = None


def _build_nc():
    nc = bacc.Bacc("TRN2", target_bir_lowering=False, debug=False,
                   num_devices=N_CORES)

    x_ext = nc.dram_tensor("x", [2 * OBS, ROWS_PC], BF16, kind="ExternalInput")
    wl1_ext = nc.dram_tensor("wl1", [2 * OBS, HIDDEN], BF16, kind="ExternalInput")
    wcat_ext = nc.dram_tensor("wcat", [N_AGENTS, 2 * OBS, HIDDEN], BF16,
                              kind="ExternalInput")
    w2a_ext = nc.dram_tensor("w2a", [HIDDEN, HIDDEN], BF16, kind="ExternalInput")
    w2b_ext = nc.dram_tensor("w2b", [HIDDEN, HIDDEN], BF16, kind="ExternalInput")
    wv_ext = nc.dram_tensor("wv", [HIDDEN, NUM_OUT], BF16, kind="ExternalInput")
    b1_ext = nc.dram_tensor("b1", [HIDDEN, 1], F32, kind="ExternalInput")
    b1o_ext = nc.dram_tensor("b1o", [HIDDEN, 1], F32, kind="ExternalInput")
    b2_ext = nc.dram_tensor("b2", [HIDDEN, 1], F32, kind="ExternalInput")
    bv_ext = nc.dram_tensor("bv", [NUM_OUT, 1], F32, kind="ExternalInput")
    y_ext = nc.dram_tensor("y", [NUM_OUT, ROWS_PC], F32, kind="ExternalOutput")

    with tile.TileContext(nc) as tc:
        with (
            tc.tile_pool(name="const", bufs=1) as cpool,
            tc.tile_pool(name="xin", bufs=4) as xpool,
            tc.tile_pool(name="act", bufs=3) as apool,
            tc.tile_pool(name="rbuf", bufs=3) as rpool,
            tc.tile_pool(name="outb", bufs=3) as opool,
            tc.tile_pool(name="ps", bufs=4, space="PSUM") as pps,
        ):
            # --- persistent weights / biases ---
            wl1 = cpool.tile([2 * OBS, HIDDEN], BF16)
            nc.sync.dma_start(wl1[:], wl1_ext[:])
            wcat = cpool.tile([2 * OBS, N_AGENTS * HIDDEN], BF16)
            for s in range(N_AGENTS):
                nc.sync.dma_start(wcat[:, s * HIDDEN:(s + 1) * HIDDEN],
                                  wcat_ext[s])
            w2a = cpool.tile([HIDDEN, HIDDEN], BF16)
            nc.sync.dma_start(w2a[:], w2a_ext[:])
            w2b = cpool.tile([HIDDEN, HIDDEN], BF16)
            nc.sync.dma_start(w2b[:], w2b_ext[:])
            wv = cpool.tile([HIDDEN, NUM_OUT], BF16)
            nc.sync.dma_start(wv[:], wv_ext[:])
            b1t = cpool.tile([HIDDEN, 1], F32)
            nc.sync.dma_start(b1t[:], b1_ext[:])
            b1ot = cpool.tile([HIDDEN, 1], F32)
            nc.sync.dma_start(b1ot[:], b1o_ext[:])
            b2t = cpool.tile([HIDDEN, 1], F32)
            nc.sync.dma_start(b2t[:], b2_ext[:])
            bvt = cpool.tile([NUM_OUT, 1], F32)
            nc.sync.dma_start(bvt[:], bv_ext[:])

            n_tt = 6 - N_FOLD

            def drain(dst, src, bias, on_act):
                """relu(src + bias) -> dst (PSUM -> SBUF)."""
                if on_act:
                    nc.scalar.activation(dst, src, AF.Relu, bias=bias)
                else:
                    nc.vector.tensor_scalar(dst, src, bias, 0.0,
                                            ALU.add, ALU.max)

            def front(pair):
                """xt DMA, layer-1 matmuls + drains, k-sum TT tree."""
                t0 = 2 * pair
                col0 = t0 * TILE_N

                xt = xpool.tile([2 * OBS, PAIR_N], BF16)
                nc.sync.dma_start(xt[:], x_ext[:, col0:col0 + PAIR_N])

                ps1 = pps.tile([HIDDEN, PAIR_N], F32, tag="ps")
                nc.tensor.matmul(ps1[:, :TILE_N], wl1[:], xt[:, :TILE_N])
                nc.tensor.matmul(ps1[:, TILE_N:], wl1[:], xt[:, TILE_N:])
                htop = apool.tile([HIDDEN, PAIR_N], BF16, tag="htop")
                drain(htop[:], ps1[:], b1t[:], HTOP_ON_ACT)

                # r layout [128, k=7, t=2, s=8, b=64]
                r = rpool.tile([HIDDEN, 7 * PAIR_N], BF16)
                r_v = r[:].rearrange("p (k t s b) -> p k t s b",
                                     k=7, t=2, s=N_AGENTS)
                for s in range(N_AGENTS):
                    w_s = wcat[:, s * HIDDEN:(s + 1) * HIDDEN]
                    ps = pps.tile([HIDDEN, PAIR_N], F32, tag="ps")
                    for ti in range(2):
                        nc.tensor.matmul(
                            ps[:, ti * TILE_N:ti * TILE_N + 7 * NB], w_s,
                            xt[:, ti * TILE_N + NB:(ti + 1) * TILE_N])
                    src = ps[:].rearrange("p (t c) -> p t c", t=2)
                    src = src[:, :, :7 * NB].rearrange("p t (k b) -> p k t b",
                                                       k=7)
                    drain(r_v[:, :, :, s, :], src, b1ot[:],
                          s not in DVE_SHIFT_SET)

                r_k = r[:].rearrange("p (k c) -> p k c", k=7)
                hbot = apool.tile([HIDDEN, PAIR_N], BF16, tag="hbot")
                with nc.allow_low_precision("bf16 partial sums"):
                    if n_tt == 0:
                        hbot = None
                    elif n_tt >= 3:
                        tmp = apool.tile([HIDDEN, PAIR_N], BF16, tag="tmp")
                        nc.vector.tensor_add(hbot[:], r_k[:, 0, :], r_k[:, 1, :])
                        nc.vector.tensor_add(tmp[:], r_k[:, 2, :], r_k[:, 3, :])
                        for k in range(4, n_tt + 1):
                            nc.vector.tensor_add(tmp[:], tmp[:], r_k[:, k, :])
                        nc.vector.tensor_add(hbot[:], hbot[:], tmp[:])
                    else:
                        nc.vector.tensor_add(hbot[:], r_k[:, 0, :], r_k[:, 1, :])
                        for k in range(2, n_tt + 1):
                            nc.vector.tensor_add(hbot[:], hbot[:], r_k[:, k, :])
                return r, hbot, htop

            def back(pair, state):
                """layer 2 + layer 3 + output DMA."""
                r, hbot, htop = state
                col0 = 2 * pair * TILE_N
                first_fold = 7 - N_FOLD if n_tt > 0 else 0
                r_kt = r[:].rearrange("p (k t c) -> p k t c", k=7, t=2)
                ps2 = pps.tile([HIDDEN, PAIR_N], F32, tag="ps")
                for ti in range(2):
                    sl = slice(ti * TILE_N, (ti + 1) * TILE_N)
                    nc.tensor.matmul(ps2[:, sl], w2a[:], htop[:, sl],
                                     start=True, stop=False)
                for ti in range(2):
                    sl = slice(ti * TILE_N, (ti + 1) * TILE_N)
                    if hbot is not None:
                        nc.tensor.matmul(ps2[:, sl], w2b[:],
                                         hbot[:, sl],
                                         start=False, stop=(N_FOLD == 0))
                    for k in range(first_fold, 7):
                        nc.tensor.matmul(ps2[:, sl], w2b[:], r_kt[:, k, ti, :],
                                         start=False, stop=(k == 6))
                h2 = apool.tile([HIDDEN, PAIR_N], BF16, tag="h2")
                drain(h2[:], ps2[:], b2t[:], H2_ON_ACT)

                ps3 = pps.tile([NUM_OUT, PAIR_N], F32, tag="ps")
                for ti in range(2):
                    sl = slice(ti * TILE_N, (ti + 1) * TILE_N)
                    nc.tensor.matmul(ps3[:, sl], wv[:], h2[:, sl])
                o = opool.tile([NUM_OUT, PAIR_N], F32)
                if O_ON_ACT:
                    nc.scalar.activation(o[:], ps3[:], AF.Identity, bias=bvt[:])
                else:
                    nc.vector.tensor_scalar_add(o[:], ps3[:], bvt[:])
                nc.sync.dma_start(y_ext[:, col0:col0 + PAIR_N], o[:])

            prev = None
            for pair in range(N_TILES // 2):
                state = front(pair)
                if prev is not None:
                    back(pair - 1, prev)
                prev = state
            back(N_TILES // 2 - 1, prev)

    nc.compile()
    return nc


def kernel(inputs, w1, b1, w1o, b1o, w2, b2, wv, bv):
    global _compiled_nc, last_exec_time_ns
    if _compiled_nc is None:
        _compiled_nc = _build_nc()
    nc = _compiled_nc

    inputs = np.asarray(inputs, dtype=np.float32)
    w1 = np.asarray(w1, dtype=np.float32)
    b1 = np.asarray(b1, dtype=np.float32)
    w1o = np.asarray(w1o, dtype=np.float32)
    b1o = np.asarray(b1o, dtype=np.float32)
    w2 = np.asarray(w2, dtype=np.float32)
    b2 = np.asarray(b2, dtype=np.float32)
    wv = np.asarray(wv, dtype=np.float32)
    bv = np.asarray(bv, dtype=np.float32)

    wl1 = np.zeros((2 * OBS, HIDDEN), dtype=NP_BF16)
    wl1[:OBS] = w1.T.astype(NP_BF16)
    wcat = np.zeros((N_AGENTS, 2 * OBS, HIDDEN), dtype=NP_BF16)
    for si in range(N_AGENTS):
        wcat[si, :OBS] = (np.roll(w1o, si + 1, axis=1).T / N_AGENTS).astype(NP_BF16)
    w2a = np.ascontiguousarray(w2[:, :HIDDEN].T).astype(NP_BF16)
    w2b = np.ascontiguousarray(w2[:, HIDDEN:].T).astype(NP_BF16)
    wvt = np.ascontiguousarray(wv.T).astype(NP_BF16)
    b1c = np.ascontiguousarray(b1[:, None]).astype(np.float32)
    b1oc = np.ascontiguousarray((b1o / N_AGENTS)[:, None]).astype(np.float32)
    b2c = np.ascontiguousarray(b2[:, None]).astype(np.float32)
    bvc = np.ascontiguousarray(bv[:, None]).astype(np.float32)

    xs = inputs.reshape(N_CORES, N_TILES, NB, N_AGENTS, OBS)
    xs_t = xs.transpose(0, 4, 1, 3, 2).reshape(N_CORES, OBS, ROWS_PC)
    in_maps = []
    for c in range(N_CORES):
        x2 = np.zeros((2 * OBS, ROWS_PC), dtype=NP_BF16)
        x2[:OBS] = xs_t[c].astype(NP_BF16)
        in_maps.append({
            "x": x2,
            "wl1": wl1, "wcat": wcat, "w2a": w2a, "w2b": w2b, "wv": wvt,
            "b1": b1c, "b1o": b1oc, "b2": b2c, "bv": bvc,
        })

    trace = bool(int(os.environ.get("BASS_KERNEL_TRACE", "0")))
    res = run_bass_kernel_spmd(nc, in_maps, list(range(N_CORES)), trace=trace)
    last_exec_time_ns = res.exec_time_ns

    y = np.stack([res.results[c]["y"] for c in range(N_CORES)])
    y = y.reshape(N_CORES, NUM_OUT, N_TILES, N_AGENTS, NB)
    out = y.transpose(0, 2, 4, 3, 1).reshape(N_CORES * ROWS_PC, NUM_OUT)
    return np.ascontiguousarray(out, dtype=np.float32)
